# revision 3
# baseline (speedup 1.0000x reference)
"""Trainium2 Bass kernel for nn_CommunicationLayer (gnn_message_passing).

Computes, for A=3 agents over batch B with feature dim D=128:
    total       = sum_a x_a                      # [1, B, D]
    mean_others = (total - x_i) / (A-1)          # [A, B, D]
    out_i       = x_i + mean_others_i @ W + b    # [A, B, D]

The problem is HBM-bandwidth bound (fp32 roofline ~562us/core for the
201MB of I/O). The correctness gate is rel_err < 2e-2, and fp16 carries
~1e-4 relative error through this computation, so all device I/O runs in
fp16: the host casts x to fp16 (and W/(A-1) to fp16), the device loads
50.3MB + stores 50.3MB per core, and the host casts the fp16 result back
to fp32. That halves the DMA roofline to ~281us/core.

Per-agent messages are formed as d_j = x_{j+1} + x_{j+2} (pairwise DVE
adds, the /(A-1) is folded into W on the host), so each agent needs just
one PE transpose + one N=128 matmul per 128-row group:

  DMA in (SP/HWDGE, fp16, 3MiB chunks, 8KiB contiguous runs)
    -> DVE pairwise adds d_j per quad (fp16, 4x perf mode)
    -> PE transpose d_j (fp16, 1 cyc/row) -> ACT copy PSUM->SBUF
    -> per group: 3 matmuls (lhsT=d_j^T, rhs=W', N=128, fp32 PSUM)
    -> DVE fused residual+evacuation: out = PSUM + x (fp16 out)
    -> per-half-chunk DMA out on the otherwise-idle GPSIMD sequencer.

Distribution: data-parallel over the batch axis across 8 NeuronCores
(no cross-device communication), weights replicated.
"""

import numpy as np

import concourse.bacc as bacc
import concourse.bass as bass  # noqa: F401
import concourse.mybir as mybir
from concourse.tile import TileContext
from concourse.masks import make_identity
from concourse.bass_utils import run_bass_kernel_spmd

A = 3
B = 524288
D = 128
NCORES = 8
BC = B // NCORES          # 65536 batch rows per core
CHUNK = 4096              # batch rows per chunk
W_PER = CHUNK // 128      # 32 rows per partition per chunk
NCHUNK = BC // CHUNK      # 16
NQUAD = W_PER // 4        # 8 quads of 4 groups per chunk

F32 = mybir.dt.float32
F16 = mybir.dt.float16


def build_bass():
    # Bacc (not plain Bass): its compile pipeline moves matmul waits onto
    # ldweights and splits >1-wait sync conditions into event semaphores.
    nc = bacc.Bacc(None, target_bir_lowering=False)

    x_ext = nc.declare_dram_parameter("x", [A, BC, D], F16, isOutput=False)
    m_ext = nc.declare_dram_parameter("m", [D, D], F16, isOutput=False)
    y_ext = nc.declare_dram_parameter("y", [A, BC, D], F16, isOutput=True)

    with TileContext(nc) as tc:
        with (
            tc.tile_pool(name="const", bufs=1) as cpool,
            tc.tile_pool(name="xin_pool", bufs=4) as in_pool,
            tc.tile_pool(name="xout_pool", bufs=4) as out_pool,
            tc.tile_pool(name="dq_pool", bufs=3) as dq_pool,
            tc.tile_pool(name="dt_pool", bufs=6) as dt_pool,
            tc.tile_pool(name="tpsum_pool", bufs=4, space="PSUM") as tpsum_pool,
            tc.tile_pool(name="mpsum_pool", bufs=4, space="PSUM") as mpsum_pool,
        ):
            ident_f = cpool.tile([128, 128], F32)
            make_identity(nc, ident_f)
            ident = cpool.tile([128, 128], F16)
            nc.scalar.copy(out=ident, in_=ident_f)

            mw = cpool.tile([D, D], F16)
            nc.sync.dma_start(out=mw, in_=m_ext[:, :])

            for c in range(NCHUNK):
                b0 = c * CHUNK
                xin = in_pool.tile([128, A * CHUNK], F16, tag="xin")
                src = x_ext[:, b0:b0 + CHUNK, :].rearrange(
                    "a (p w) d -> p a (w d)", p=128
                )
                nc.sync.dma_start(
                    out=xin.rearrange("p (a f) -> p a f", a=A), in_=src
                )
                xin4 = xin.rearrange("p (a w d) -> p a w d", a=A, d=D)

                for h in range(2):
                    # Per-half-chunk output tile: its store DMA (issued on
                    # the otherwise-idle GPSIMD sequencer) waits only on this
                    # half's 16 residual adds, so the SP sequencer's load
                    # stream never blocks behind store data dependencies.
                    xoh = out_pool.tile([128, A * 16 * D], F16, tag="xout")
                    xoh4 = xoh.rearrange("p (a w d) -> p a w d", a=A, d=D)
                    for q in range(4 * h, 4 * h + 4):
                        # Pairwise agent sums for this quad: d_j = sum of the
                        # other two agents' rows (the 1/(A-1) is folded into
                        # the weights host-side).
                        dq = dq_pool.tile([128, A * 512], F16, tag="dq")
                        for j in range(A):
                            a1, a2 = (j + 1) % A, (j + 2) % A
                            nc.vector.tensor_add(
                                out=dq[:, j * 512:(j + 1) * 512],
                                in0=xin[:, a1 * CHUNK + q * 512:
                                        a1 * CHUNK + (q + 1) * 512],
                                in1=xin[:, a2 * CHUNK + q * 512:
                                        a2 * CHUNK + (q + 1) * 512],
                            )

                        # Transpose each d_j (4 groups) into feature-major.
                        dts = []
                        for j in range(A):
                            tp = tpsum_pool.tile([128, 512], F16, tag="tp")
                            for g4 in range(4):
                                nc.tensor.transpose(
                                    tp[:, g4 * 128:(g4 + 1) * 128],
                                    dq[:, j * 512 + g4 * 128:
                                        j * 512 + (g4 + 1) * 128],
                                    ident,
                                )
                            dt = dt_pool.tile([128, 512], F16, tag="dt")
                            nc.scalar.copy(out=dt, in_=tp)
                            dts.append(dt)

                        for g4 in range(4):
                            g = q * 4 + g4
                            ps = mpsum_pool.tile([128, A * D], F32, tag="ps")
                            ps_r = ps.rearrange("p (i d) -> p i d", d=D)
                            for j in range(A):
                                nc.tensor.matmul(
                                    ps_r[:, j, :],
                                    lhsT=dts[j][:, g4 * 128:(g4 + 1) * 128],
                                    rhs=mw,
                                    start=True,
                                    stop=True,
                                )
                            # Fused residual add + PSUM->SBUF evacuation.
                            nc.vector.tensor_add(
                                out=xoh4[:, :, g - 16 * h, :],
                                in0=ps_r,
                                in1=xin4[:, :, g, :],
                            )

                    dst = y_ext[:, b0:b0 + CHUNK, :].rearrange(
                        "a (p w) d -> p a w d", p=128
                    )[:, :, 16 * h:16 * h + 16, :]
                    nc.gpsimd.dma_start(out=dst, in_=xoh4)

    # Bacc defers register allocation to its compile() pass (run by
    # finalize); the PJRT exec path serializes nc as-is, so finalize here.
    nc.finalize()
    return nc


def run(inputs, trace=False):
    """Build, compile, and run on 8 cores. Returns (full_output, results_obj)."""
    agent_states = np.asarray(inputs["agent_states"], dtype=np.float32)
    W = np.asarray(inputs["W"], dtype=np.float32)
    b = np.asarray(inputs["b"], dtype=np.float32)

    m_host = np.ascontiguousarray((W * (1.0 / (A - 1))).astype(np.float16))
    x16 = agent_states.astype(np.float16)

    nc = build_bass()

    in_maps = []
    for i in range(NCORES):
        shard = np.ascontiguousarray(x16[:, i * BC:(i + 1) * BC, :])
        in_maps.append({"x": shard, "m": m_host})

    res = run_bass_kernel_spmd(nc, in_maps, list(range(NCORES)), trace=trace)

    out = np.concatenate([r["y"] for r in res.results], axis=1).astype(np.float32)
    if np.any(b):
        out = out + b.reshape(1, 1, D)
    return out, res


def kernel(**inputs):
    out, _ = run(inputs, trace=False)
    return out
